# revision 1
# baseline (speedup 1.0000x reference)
"""Trainium2 Bass kernel for nn_CustomConvLayer (dynamic per-sample conv).

Sharding: pure data parallel over batch B=32 across 8 NeuronCores (4
samples per core). Small synthesis networks are replicated per core.

Per sample, on device:
  wm-embedding MLP -> per-channel modulation wm_coff  (tiny matmuls)
  t0 = avg_pool2(wm_coff * x)                         (DVE 2x2 window reduce)
  tower: 3 convs (lrelu) -> t3                        (9-tap matmul conv, bf16)
  4 coeff heads + attention head (conv+GAP)           (bf16 matmuls + accum)
  w_dyn synthesis (4 experts)                         (DVE scalar_tensor_tensor)
  main conv: out = conv(x, w_dyn*wm_coff), 3x3 pad 1  (fp32r matmuls, 9 taps
                                                       accumulated in PSUM)
The wm_coff modulation of x is folded into t0 (tower side) and into the
synthesized weights (main conv side), so the full-res image is never
rescaled.
"""

import sys

if "/opt/trn_rl_repo" not in sys.path:
    sys.path.insert(0, "/opt/trn_rl_repo")

import numpy as np
import ml_dtypes
from contextlib import ExitStack

import concourse.bass as bass
import concourse.bacc as bacc
import concourse.tile as tile
from concourse import mybir
from concourse.bass_utils import run_bass_kernel_spmd

F32 = mybir.dt.float32
F32R = mybir.dt.float32r
BF16 = mybir.dt.bfloat16
U32 = mybir.dt.uint32
AF = mybir.ActivationFunctionType
OP = mybir.AluOpType


class Cfg:
    def __init__(self, BL=4, Cin=128, H=128, W=128, n_cores=8, main_bf16=True):
        self.BL, self.Cin, self.H, self.W, self.n_cores = BL, Cin, H, W, n_cores
        self.main_bf16 = main_bf16
        self.Cout = 64
        self.Hp, self.Wp = H + 2, W + 2
        self.NPAD = self.Hp * self.Wp
        # pooled size and tower conv output sizes
        self.P, self.PW = H // 2, W // 2
        self.c1h, self.c1w = self.P - 2, self.PW - 2
        self.c2h, self.c2w = (self.c1h - 3) // 2 + 1, (self.c1w - 3) // 2 + 1
        self.c3h, self.c3w = (self.c2h - 3) // 2 + 1, (self.c2w - 3) // 2 + 1
        self.hh, self.hw = (self.c3h - 3) // 2 + 1, (self.c3w - 3) // 2 + 1
        self.gapn = self.hh * self.hw
        # main conv row groups: pairs of row-groups share one PSUM bank
        self.RPG = 512 // W          # rows per row-group (N = RPG*W = 512)
        assert H % (2 * self.RPG) == 0
        self.NRG = H // self.RPG
        self.NPAIR = self.NRG // 2
        self.POUT = min(4, self.NPAIR)   # psum-pairs per output staging tile
        assert self.NPAIR % self.POUT == 0
        self.NOUT = self.NPAIR // self.POUT
        # ---- const blob layout (uint32 columns) ----
        o = 0

        def take(n):
            nonlocal o
            r = (o, o + n)
            o += n
            return r

        self.EXP = take(4 * 9 * self.Cout)        # expertT f32 [Cin, 4*9*64]
        self.WM2 = take(self.Cin)                 # wm_w2T f32 [Cin, Cin]
        self.WM1 = take(self.Cin)                 # wm_w1T f32 [32, Cin]
        self.WMT = take(BL)                       # wm.T f32 [32, BL]
        self.AFW = take(4)                        # att_fw.T f32 [Cin, 4]
        self.W1 = take(9 * self.Cout // 2)        # w1T bf16 [Cin, 9*64]
        self.W2 = take(9 * self.Cout // 2)        # w2T bf16 [64, 9*64]
        self.W3 = take(9 * self.Cout // 2)        # w3T bf16 [64, 9*64]
        self.HD = take(5 * 9 * self.Cin // 2)     # headT bf16 [64, 5*9*128]
        self.TB = take(3)                         # tower biases f32 [64, 3]
        self.HB = take(5)                         # head biases f32 [128, 5]
        self.AFB = take(4)                        # att_fb/gapn f32 [1, 4]
        self.WB1 = take(1)                        # wm_b1 f32 [128, 1]
        self.WB2 = take(1)                        # wm_b2 f32 [128, 1]
        self.ONE = take(self.Cin)                 # ones f32 [1, Cin]
        self.NBLOB = o


def _pack_f32(dst, col, arr):
    """Pack f32 array [r, c] into dst u32 blob at column col."""
    a = np.ascontiguousarray(arr, dtype=np.float32)
    dst[: a.shape[0], col : col + a.shape[1]] = a.view(np.uint32)


def _pack_bf16(dst, col, arr):
    a = np.ascontiguousarray(arr, dtype=ml_dtypes.bfloat16)
    u16 = a.view(np.uint16)
    u32 = (u16[:, 1::2].astype(np.uint32) << 16) | u16[:, 0::2].astype(np.uint32)
    dst[: a.shape[0], col : col + u32.shape[1]] = u32


def make_blob(cfg, wm_core, wm_w1, wm_b1, wm_w2, wm_b2, tr_w1, tr_b1, tr_w2,
              tr_b2, tr_w3, tr_b3, t1_w, t1_b, t2_w, t2_b, t3_w, t3_b, t4_w,
              t4_b, att_cw, att_cb, att_fw, att_fb, expert_w):
    """Pack all weights + this core's wm rows into one [128, NBLOB] u32 blob."""
    blob = np.zeros((128, cfg.NBLOB), np.uint32)
    # expertT[i, e, kh, kw, o] from expert_w[0][e, o, i, kh, kw]
    expT = expert_w[0].transpose(2, 0, 3, 4, 1).reshape(cfg.Cin, -1)
    _pack_f32(blob, cfg.EXP[0], expT)
    _pack_f32(blob, cfg.WM2[0], wm_w2.T)
    _pack_f32(blob, cfg.WM1[0], wm_w1.T)
    _pack_f32(blob, cfg.WMT[0], wm_core.T)
    _pack_f32(blob, cfg.AFW[0], att_fw.T)
    # conv weights [o, i, kh, kw] -> [i, kh, kw, o]
    _pack_bf16(blob, cfg.W1[0], tr_w1.transpose(1, 2, 3, 0).reshape(cfg.Cin, -1))
    _pack_bf16(blob, cfg.W2[0], tr_w2.transpose(1, 2, 3, 0).reshape(64, -1))
    _pack_bf16(blob, cfg.W3[0], tr_w3.transpose(1, 2, 3, 0).reshape(64, -1))
    hd = np.concatenate(
        [w.transpose(1, 2, 3, 0).reshape(64, -1) for w in (t1_w, t2_w, t3_w, t4_w, att_cw)],
        axis=1,
    )
    _pack_bf16(blob, cfg.HD[0], hd)
    _pack_f32(blob, cfg.TB[0], np.stack([tr_b1, tr_b2, tr_b3], 1))
    _pack_f32(blob, cfg.HB[0], np.stack([t1_b, t2_b, t3_b, t4_b, att_cb], 1))
    _pack_f32(blob, cfg.AFB[0], (att_fb / cfg.gapn)[None, :])
    _pack_f32(blob, cfg.WB1[0], wm_b1[:, None])
    _pack_f32(blob, cfg.WB2[0], wm_b2[:, None])
    _pack_f32(blob, cfg.ONE[0], np.ones((1, cfg.Cin), np.float32))
    return blob


def build_nc(cfg):
    nc = bacc.Bacc()
    Cin, Cout, H, W = cfg.Cin, cfg.Cout, cfg.H, cfg.W
    xdt = BF16 if cfg.main_bf16 else F32R
    xin = nc.declare_dram_parameter("x", [cfg.BL, Cin, cfg.NPAD], xdt, isOutput=False)
    wblob = nc.declare_dram_parameter("wblob", [128, cfg.NBLOB], U32, isOutput=False)
    y = nc.declare_dram_parameter("y", [cfg.BL, Cout, H, W], F32, isOutput=True)

    with tile.TileContext(nc) as tc, ExitStack() as ctx:
        cpool = ctx.enter_context(tc.tile_pool(name="consts", bufs=1))
        xpool = ctx.enter_context(tc.tile_pool(name="xpad", bufs=1))
        dpool = ctx.enter_context(tc.tile_pool(name="data", bufs=1))
        spool = ctx.enter_context(tc.tile_pool(name="smalls", bufs=2))
        ypool = ctx.enter_context(tc.tile_pool(name="synth", bufs=2))
        wpool = ctx.enter_context(tc.tile_pool(name="wdyn", bufs=2))
        opool = ctx.enter_context(tc.tile_pool(name="outsb", bufs=2))
        mpsum = ctx.enter_context(tc.tile_pool(name="mpsum", bufs=3, space="PSUM"))
        tpsum = ctx.enter_context(tc.tile_pool(name="tpsum", bufs=2, space="PSUM"))
        hpsum = ctx.enter_context(tc.tile_pool(name="hpsum", bufs=2, space="PSUM"))

        blob = cpool.tile([128, cfg.NBLOB], U32)
        nc.gpsimd.dma_start(blob[:], wblob[:])

        def bl(rng, nrows=128, dt=F32):
            ap = blob[0:nrows, rng[0]: rng[1]]
            return ap.bitcast(dt)

        expT = bl(cfg.EXP)
        wm_w2T = bl(cfg.WM2)
        wm_w1T = bl(cfg.WM1, 32)
        wmT = bl(cfg.WMT, 32)
        att_fwT = bl(cfg.AFW)
        w1T = bl(cfg.W1, 128, BF16)
        w2T = bl(cfg.W2, 64, BF16)
        w3T = bl(cfg.W3, 64, BF16)
        headT = bl(cfg.HD, 64, BF16)
        tr_b = bl(cfg.TB, 64)
        head_b = bl(cfg.HB)
        att_fb = bl(cfg.AFB, 1)
        wm_b1 = bl(cfg.WB1)
        wm_b2 = bl(cfg.WB2)
        ones_row = bl(cfg.ONE, 1)
        # wm-embedding scratch (written once, read per-sample)
        wmx = cpool.tile([128, 3 * cfg.BL], F32)
        hT = wmx[:, 0 : cfg.BL]
        wmc = wmx[:, cfg.BL : 2 * cfg.BL]
        wq = wmx[:, 2 * cfg.BL : 3 * cfg.BL]

        XDT = BF16 if cfg.main_bf16 else F32R
        nxp = 3 if cfg.main_bf16 else 2
        # padded-image buffers (borders arrive pre-zeroed from the host)
        xpads = [
            xpool.tile([128, cfg.NPAD], XDT, tag=f"xp{i}", name=f"xp{i}")
            for i in range(nxp)
        ]
        xvs = [xp[:].rearrange("p (r c) -> p r c", c=cfg.Wp) for xp in xpads]
        if cfg.main_bf16:
            xv_pool = xvs
        else:
            xv_pool = [
                xp[:].bitcast(F32).rearrange("p (r c) -> p r c", c=cfg.Wp)
                for xp in xpads
            ]

        scr = dpool.tile([128, 512], F32, tag="scr")
        n_t1, n_t2, n_t3 = cfg.c1h * cfg.c1w, cfg.c2h * cfg.c2w, cfg.c3h * cfg.c3w
        tower = dpool.tile([64, n_t1 + n_t2 + n_t3], BF16, tag="tower")
        t1v = tower[:, 0:n_t1].rearrange("p (r c) -> p r c", c=cfg.c1w)
        t2v = tower[:, n_t1 : n_t1 + n_t2].rearrange("p (r c) -> p r c", c=cfg.c2w)
        t3v = tower[:, n_t1 + n_t2 :].rearrange("p (r c) -> p r c", c=cfg.c3w)

        def lrelu_inplace(ap, alpha):
            nc.vector.scalar_tensor_tensor(
                ap, ap, float(alpha), ap, op0=OP.mult, op1=OP.max
            )

        # ---- wm embedding -> wm_coff.T [Cin, BL] (once, all samples) ----
        ps = hpsum.tile([128, cfg.BL], F32, tag="hps")
        nc.tensor.matmul(ps[:], wm_w1T, wmT, start=True, stop=True)
        nc.scalar.activation(hT, ps[:], AF.Identity, bias=wm_b1)
        lrelu_inplace(hT, 0.2)
        ps = hpsum.tile([128, cfg.BL], F32, tag="hps")
        nc.tensor.matmul(ps[:], wm_w2T, hT, start=True, stop=True)
        nc.scalar.activation(wmc, ps[:], AF.Identity, bias=wm_b2)
        nc.vector.tensor_scalar_mul(wq, wmc, 0.25)

        prows = max(1, 512 // cfg.PW)   # pooled rows per pooling chunk
        assert cfg.P % prows == 0

        t0s, wdyns = {}, {}

        # ---------- per-sample stage emitters ----------
        def emit_dma(s):
            # chunked so pooling/conv can start before the full image lands
            xp = xpads[s % nxp]
            step = (cfg.Hp + 3) // 4 * cfg.Wp
            for c0 in range(0, cfg.NPAD, step):
                c1 = min(cfg.NPAD, c0 + step)
                nc.gpsimd.dma_start(xp[:, c0:c1], xin[s, :, c0:c1])

        def emit_pool(s):
            t0 = dpool.tile([128, cfg.P * cfg.PW], BF16, tag="t0", bufs=2,
                            name=f"t0_{s}")
            t0s[s] = t0
            xvp = xv_pool[s % nxp]
            for q in range(cfg.P // prows):
                rows = xvp[:, 1 + 2 * prows * q : 1 + 2 * prows * (q + 1),
                           1 : cfg.W + 1]
                blk = rows.rearrange("p (y a) (x b) -> p y x a b", a=2, b=2)
                sc = scr[:, 0 : prows * cfg.PW].rearrange(
                    "p (y x) -> p y x", x=cfg.PW
                )
                nc.vector.tensor_reduce(sc, blk, axis=mybir.AxisListType.XY,
                                        op=OP.add)
                nc.vector.tensor_scalar_mul(
                    t0[:, prows * cfg.PW * q : prows * cfg.PW * (q + 1)],
                    scr[:, 0 : prows * cfg.PW],
                    wq[:, s : s + 1],
                )

        def emit_conv1(s):
            t0v = t0s.pop(s)[:].rearrange("p (r c) -> p r c", c=cfg.PW)
            rb = max(1, min(cfg.c1h, 512 // cfg.c1w))
            for y0 in range(0, cfg.c1h, rb):
                nb = min(rb, cfg.c1h - y0)
                ps = tpsum.tile([64, nb * cfg.c1w], F32, tag="tps")
                for ky in range(3):
                    for kx in range(3):
                        nc.tensor.matmul(
                            ps[:],
                            w1T[:, (ky * 3 + kx) * 64 : (ky * 3 + kx + 1) * 64],
                            t0v[:, y0 + ky : y0 + ky + nb, kx : kx + cfg.c1w],
                            start=(ky == 0 and kx == 0),
                            stop=(ky == 2 and kx == 2),
                        )
                dst = t1v[:, y0 : y0 + nb, :]
                nc.scalar.activation(dst, ps[:], AF.Identity, bias=tr_b[:, 0:1])
                lrelu_inplace(dst, 0.01)

        def emit_conv23(s):
            rb = max(1, min(cfg.c2h, 512 // cfg.c2w))
            for y0 in range(0, cfg.c2h, rb):
                nb = min(rb, cfg.c2h - y0)
                ps = tpsum.tile([64, nb * cfg.c2w], F32, tag="tps")
                for ky in range(3):
                    for kx in range(3):
                        rhs = t1v[:, 2 * y0 + ky : 2 * y0 + ky + 2 * nb : 2,
                                  kx : kx + 2 * cfg.c2w - 1 : 2]
                        nc.tensor.matmul(
                            ps[:],
                            w2T[:, (ky * 3 + kx) * 64 : (ky * 3 + kx + 1) * 64],
                            rhs,
                            start=(ky == 0 and kx == 0),
                            stop=(ky == 2 and kx == 2),
                        )
                dst = t2v[:, y0 : y0 + nb, :]
                nc.scalar.activation(dst, ps[:], AF.Identity, bias=tr_b[:, 1:2])
                lrelu_inplace(dst, 0.01)

            ps = tpsum.tile([64, cfg.c3h * cfg.c3w], F32, tag="tps")
            for ky in range(3):
                for kx in range(3):
                    rhs = t2v[:, ky : ky + 2 * cfg.c3h - 1 : 2,
                              kx : kx + 2 * cfg.c3w - 1 : 2]
                    nc.tensor.matmul(
                        ps[:],
                        w3T[:, (ky * 3 + kx) * 64 : (ky * 3 + kx + 1) * 64],
                        rhs,
                        start=(ky == 0 and kx == 0),
                        stop=(ky == 2 and kx == 2),
                    )
            dst = t3v[:, :, :]
            nc.scalar.activation(dst, ps[:], AF.Identity, bias=tr_b[:, 2:3])
            lrelu_inplace(dst, 0.01)

        def emit_heads_att_synth(s):
            sm = spool.tile([128, 64], F32, tag="sm", name=f"sm_{s}")
            a_sb = sm[:, 0:1]
            att_row = sm[0:1, 4:8]
            att_bc = sm[:, 8:12]
            cc = sm[:, 12:16]
            gap = sm[:, 16:24]
            hscr = sm[:, 24:42].bitcast(BF16)[:, 0 : cfg.gapn]
            for h in range(5):
                ps = hpsum.tile([128, cfg.gapn], F32, tag="hps")
                for ky in range(3):
                    for kx in range(3):
                        rhs = t3v[:, ky : ky + 2 * cfg.hh - 1 : 2,
                                  kx : kx + 2 * cfg.hw - 1 : 2]
                        idx = h * 9 + ky * 3 + kx
                        nc.tensor.matmul(
                            ps[:],
                            headT[:, idx * 128 : (idx + 1) * 128],
                            rhs,
                            start=(ky == 0 and kx == 0),
                            stop=(ky == 2 and kx == 2),
                        )
                nc.scalar.activation(
                    hscr, ps[:], AF.Identity, bias=head_b[:, h : h + 1],
                    accum_out=gap[:, h : h + 1],
                )

            # attention: a = lrelu(gap4/gapn); att = (a@att_fwT + fb)/gapn
            nc.scalar.activation(a_sb, gap[:, 4:5], AF.Copy, scale=1.0 / cfg.gapn)
            lrelu_inplace(a_sb, 0.01)
            ps = hpsum.tile([1, 4], F32, tag="hps")
            nc.tensor.matmul(ps[:], a_sb, att_fwT, start=True, stop=True)
            nc.vector.scalar_tensor_tensor(
                att_row, ps[:], 1.0 / cfg.gapn, att_fb, op0=OP.mult, op1=OP.add
            )
            ps = hpsum.tile([128, 4], F32, tag="hps")
            nc.tensor.matmul(ps[:], ones_row, att_row, start=True, stop=True)
            nc.scalar.activation(att_bc, ps[:], AF.Copy)
            nc.vector.tensor_mul(cc, att_bc, gap[:, 0:4])

            # synthesize w_dynT[i, (kh kw o)], fold in wm_coff
            A = ypool.tile([128, 9 * 64], F32, tag="synA", name=f"synA_{s}")
            Bt = ypool.tile([128, 9 * 64], F32, tag="synB", name=f"synB_{s}")
            wdyn = wpool.tile([128, 9 * 64], XDT, tag="wdyn", name=f"wdyn_{s}")
            wdyns[s] = wdyn
            nc.vector.tensor_scalar_mul(A[:], expT[:, 0:576], cc[:, 0:1])
            nc.vector.scalar_tensor_tensor(
                Bt[:], expT[:, 576:1152], cc[:, 1:2], A[:], op0=OP.mult,
                op1=OP.add,
            )
            nc.vector.scalar_tensor_tensor(
                A[:], expT[:, 1152:1728], cc[:, 2:3], Bt[:], op0=OP.mult,
                op1=OP.add,
            )
            nc.vector.scalar_tensor_tensor(
                Bt[:], expT[:, 1728:2304], cc[:, 3:4], A[:], op0=OP.mult,
                op1=OP.add,
            )
            nc.vector.tensor_scalar_mul(wdyn[:], Bt[:], wmc[:, s : s + 1])

        def emit_main_group(s, q):
            xv = xvs[s % nxp]
            wdyn = wdyns[s]
            out_t = opool.tile([128, cfg.POUT * 512], F32, tag="outsb",
                               name=f"out_{s}_{q}")
            for j in range(cfg.POUT):
                pair = q * cfg.POUT + j
                if cfg.main_bf16:
                    # even/odd row-groups stream concurrently into the two
                    # PE column groups (tile_position (0,0) / (0,64))
                    ps = mpsum.tile([128, 512], F32, tag="mps")
                    for ky in range(3):
                        for kx in range(3):
                            for half in range(2):
                                y0 = (2 * pair + half) * cfg.RPG
                                nc.tensor.matmul(
                                    ps[half * 64 : half * 64 + 64, :],
                                    wdyn[:, (ky * 3 + kx) * 64 : (ky * 3 + kx + 1) * 64],
                                    xv[:, y0 + ky : y0 + ky + cfg.RPG,
                                       kx : kx + cfg.W],
                                    start=(ky == 0 and kx == 0),
                                    stop=(ky == 2 and kx == 2),
                                )
                    nc.scalar.activation(
                        out_t[:, j * 512 : (j + 1) * 512], ps[:], AF.Copy
                    )
                else:
                    for half in range(2):
                        y0 = (2 * pair + half) * cfg.RPG
                        ps = mpsum.tile([64, 512], F32, tag="mps")
                        for ky in range(3):
                            for kx in range(3):
                                nc.tensor.matmul(
                                    ps[:],
                                    wdyn[:, (ky * 3 + kx) * 64 : (ky * 3 + kx + 1) * 64],
                                    xv[:, y0 + ky : y0 + ky + cfg.RPG,
                                       kx : kx + cfg.W],
                                    start=(ky == 0 and kx == 0),
                                    stop=(ky == 2 and kx == 2),
                                )
                        nc.scalar.activation(
                            out_t[half * 64 : half * 64 + 64,
                                  j * 512 : (j + 1) * 512],
                            ps[:], AF.Copy,
                        )
            yv = y[s].rearrange("c (j r) x -> c j r x", r=2 * cfg.RPG)
            jj = q * cfg.POUT
            for hf in range(2):
                dst = yv[:, jj : jj + cfg.POUT,
                         hf * cfg.RPG : (hf + 1) * cfg.RPG, :]
                nc.gpsimd.dma_start(dst, out_t[hf * 64 : hf * 64 + 64, :])
            if q == cfg.NOUT - 1:
                wdyns.pop(s)

        # ---------- software pipeline ----------
        # prologue: samples 0 (and 1) fully up to synth before main(0)
        emit_dma(0)
        emit_pool(0)
        if cfg.BL > 1:
            emit_dma(1)
        emit_conv1(0)
        emit_conv23(0)
        emit_heads_att_synth(0)
        if cfg.BL > 1:
            emit_pool(1)

        # stage k of sample s+1 (or s+2 for dma/pool) after main group q=k
        def stage_after(s, q):
            if q == min(0, cfg.NOUT - 1):
                if s + 2 < cfg.BL and nxp >= 3:
                    emit_dma(s + 2)
                if s + 1 < cfg.BL:
                    emit_conv1(s + 1)
            if q == min(1, cfg.NOUT - 1):
                if s + 1 < cfg.BL:
                    emit_conv23(s + 1)
            if q == min(2, cfg.NOUT - 1):
                if s + 1 < cfg.BL:
                    emit_heads_att_synth(s + 1)
            if q == cfg.NOUT - 1:
                if s + 2 < cfg.BL and nxp < 3:
                    emit_dma(s + 2)
                if s + 2 < cfg.BL:
                    emit_pool(s + 2)

        for s in range(cfg.BL):
            for q in range(cfg.NOUT):
                emit_main_group(s, q)
                stage_after(s, q)

    return nc


_NC_CACHE = {}
TRACE = False       # set by test harness to collect an NTFF profile
TRACE_DIR = None    # where to leave the NTFF/perfetto artifacts
LAST_RESULT = None  # BassKernelResults of the most recent kernel() call


def _get_nc(cfg):
    key = (cfg.BL, cfg.Cin, cfg.H, cfg.W, cfg.main_bf16)
    if key not in _NC_CACHE:
        nc = build_nc(cfg)
        if not nc.is_finalized():
            nc.finalize()
        _NC_CACHE[key] = nc
    return _NC_CACHE[key]


def pad_images(cfg, x):
    """[n, Cin, H, W] -> zero-padded flat [n, Cin, Hp*Wp]."""
    n = x.shape[0]
    dt = ml_dtypes.bfloat16 if cfg.main_bf16 else np.float32
    xp = np.zeros((n, cfg.Cin, cfg.Hp, cfg.Wp), dt)
    xp[:, :, 1 : cfg.H + 1, 1 : cfg.W + 1] = x.astype(dt)
    return xp.reshape(n, cfg.Cin, cfg.NPAD)


MAIN_BF16 = True   # main conv in bf16 (fast) vs float32r (more precise)


def kernel(**inputs):
    x = np.asarray(inputs["x"], np.float32)
    B, Cin, H, W = x.shape
    cfg = Cfg(BL=B // 8, Cin=Cin, H=H, W=W, main_bf16=MAIN_BF16)
    nc = _get_nc(cfg)
    wnames = [
        "wm_w1", "wm_b1", "wm_w2", "wm_b2", "tr_w1", "tr_b1", "tr_w2", "tr_b2",
        "tr_w3", "tr_b3", "t1_w", "t1_b", "t2_w", "t2_b", "t3_w", "t3_b",
        "t4_w", "t4_b", "att_cw", "att_cb", "att_fw", "att_fb", "expert_w",
    ]
    ws = {k: np.asarray(inputs[k], np.float32) for k in wnames}
    wm = np.asarray(inputs["wm"], np.float32)
    in_maps = []
    for c in range(8):
        sl = slice(c * cfg.BL, (c + 1) * cfg.BL)
        blob = make_blob(cfg, wm[sl], **ws)
        in_maps.append({"x": pad_images(cfg, x[sl]), "wblob": blob})
    global LAST_RESULT
    kw = {"tmpdir": TRACE_DIR} if (TRACE and TRACE_DIR) else {}
    res = run_bass_kernel_spmd(nc, in_maps, list(range(8)), trace=TRACE, **kw)
    LAST_RESULT = res
    return np.concatenate([res.results[c]["y"] for c in range(8)], axis=0)



# revision 7
# speedup vs baseline: 1.2729x; 1.2729x over previous
"""Trainium2 Bass kernel for nn_CustomConvLayer (dynamic per-sample conv).

Sharding: pure data parallel over batch B=32 across 8 NeuronCores (4
samples per core). Small synthesis networks are replicated per core.

Per sample, on device:
  wm-embedding MLP -> per-channel modulation wm_coff  (tiny matmuls)
  t0 = avg_pool2(x)                                   (two DVE tensor_tensor
                                                       adds: vertical pair sum
                                                       in 2x bf16 mode, then
                                                       strided horizontal sum)
  tower: 3 convs (ACT-fused lrelu) -> t3              (9-tap matmul conv, bf16;
                                                       wm_coff*0.25 folded into
                                                       per-sample conv1 weights)
  4 coeff heads + attention head (conv+GAP)           (bf16 matmuls + accum)
  w_dyn synthesis (4 experts)                         (DVE scalar_tensor_tensor)
  main conv: out = conv(x, w_dyn*wm_coff), 3x3 pad 1  (bf16 matmuls, PSUM accum)
Tower activations t1/t2/t3 are stored column-deinterleaved (even/odd x)
so the stride-2 convs read contiguous rhs. Output y is written bf16 and
upcast to f32 on the host.
"""

import sys

if "/opt/trn_rl_repo" not in sys.path:
    sys.path.insert(0, "/opt/trn_rl_repo")

import numpy as np
import ml_dtypes
from contextlib import ExitStack

import concourse.bass as bass
import concourse.bacc as bacc
import concourse.tile as tile
from concourse import mybir
from concourse.bass_utils import run_bass_kernel_spmd

F32 = mybir.dt.float32
BF16 = mybir.dt.bfloat16
U32 = mybir.dt.uint32
AF = mybir.ActivationFunctionType
OP = mybir.AluOpType


class Cfg:
    def __init__(self, BL=4, Cin=128, H=128, W=128, n_cores=8):
        self.BL, self.Cin, self.H, self.W, self.n_cores = BL, Cin, H, W, n_cores
        self.Cout = 64
        self.Hp, self.Wp = H + 2, W + 2
        self.NPAD = self.Hp * self.Wp
        # pooled size and tower conv output sizes
        self.P, self.PW = H // 2, W // 2
        self.c1h, self.c1w = self.P - 2, self.PW - 2
        self.c2h, self.c2w = (self.c1h - 3) // 2 + 1, (self.c1w - 3) // 2 + 1
        self.c3h, self.c3w = (self.c2h - 3) // 2 + 1, (self.c2w - 3) // 2 + 1
        self.hh, self.hw = (self.c3h - 3) // 2 + 1, (self.c3w - 3) // 2 + 1
        self.gapn = self.hh * self.hw
        # x DMA / pooling chunk rows (padded-image rows per chunk)
        self.NCHUNK = 4
        self.prows = self.P // self.NCHUNK          # pooled rows per chunk
        # dma chunks must cover pool chunk k's rows [2*prows*k+1, +2*prows+1)
        r0 = 2 * self.prows + 2
        self.dma_rows = [0, r0]
        while self.dma_rows[-1] < self.Hp:
            self.dma_rows.append(min(self.Hp, self.dma_rows[-1] + 2 * self.prows))
        # main conv row groups: pairs of row-groups share one PSUM bank
        self.RPG = 512 // W          # rows per row-group (N = RPG*W = 512)
        assert H % (2 * self.RPG) == 0
        self.NRG = H // self.RPG
        self.NPAIR = self.NRG // 2
        self.POUT = min(4, self.NPAIR)   # psum-pairs per output staging tile
        assert self.NPAIR % self.POUT == 0
        self.NOUT = self.NPAIR // self.POUT
        # ---- const blob layout (uint32 columns) ----
        o = 0

        def take(n):
            nonlocal o
            r = (o, o + n)
            o += n
            return r

        # early: wm embedding + conv1 weights
        self.WMT = take(BL)                       # wm.T f32 [32, BL]
        self.WM1 = take(self.Cin)                 # wm_w1T f32 [32, 128]
        self.WM2 = take(self.Cin)                 # wm_w2T f32 [128, 128]
        self.WB1 = take(1)                        # wm_b1 f32 [128, 1]
        self.WB2 = take(1)                        # wm_b2 f32 [128, 1]
        self.W1 = take(9 * self.Cout // 2)        # w1T bf16 [128, 9*64]
        self.TB = take(3)                         # tower biases f32 [64, 3]
        self.EARLY = o
        # mid: rest of the tower + heads
        self.W2 = take(9 * self.Cout // 2)        # w2T bf16 [64, 9*64]
        self.W3 = take(9 * self.Cout // 2)        # w3T bf16 [64, 9*64]
        self.MID1 = o
        self.HD = take(5 * 9 * self.Cin // 2)     # headT bf16 [64, 5*9*128]
        self.MID2 = self.HD[0] + 2 * 9 * self.Cin // 2   # after heads 0-1
        self.HB = take(5)                         # head biases f32 [128, 5]
        self.AFW = take(4)                        # att_fw.T f32 [128, 4]
        self.AFB = take(4)                        # att_fb/gapn f32 [1, 4]
        self.ONE = take(self.Cin)                 # ones f32 [1, 128]
        self.MID = o
        # late: expert weights
        self.EXP = take(4 * 9 * self.Cout // 2)   # expertT bf16 [128, 4*9*64]
        self.NBLOB = o


def _pack_f32(dst, col, arr):
    """Pack f32 array [r, c] into dst u32 blob at column col."""
    a = np.ascontiguousarray(arr, dtype=np.float32)
    dst[: a.shape[0], col : col + a.shape[1]] = a.view(np.uint32)


def _pack_bf16(dst, col, arr):
    a = np.ascontiguousarray(arr, dtype=ml_dtypes.bfloat16)
    u16 = a.view(np.uint16)
    u32 = (u16[:, 1::2].astype(np.uint32) << 16) | u16[:, 0::2].astype(np.uint32)
    dst[: a.shape[0], col : col + u32.shape[1]] = u32


def make_blob(cfg, wm_core, wm_w1, wm_b1, wm_w2, wm_b2, tr_w1, tr_b1, tr_w2,
              tr_b2, tr_w3, tr_b3, t1_w, t1_b, t2_w, t2_b, t3_w, t3_b, t4_w,
              t4_b, att_cw, att_cb, att_fw, att_fb, expert_w):
    """Pack all weights + this core's wm rows into one [128, NBLOB] u32 blob."""
    blob = np.zeros((128, cfg.NBLOB), np.uint32)
    _pack_f32(blob, cfg.WMT[0], wm_core.T)
    _pack_f32(blob, cfg.WM1[0], wm_w1.T)
    _pack_f32(blob, cfg.WM2[0], wm_w2.T)
    _pack_f32(blob, cfg.WB1[0], wm_b1[:, None])
    _pack_f32(blob, cfg.WB2[0], wm_b2[:, None])
    # conv weights [o, i, kh, kw] -> [i, kh, kw, o]
    _pack_bf16(blob, cfg.W1[0], tr_w1.transpose(1, 2, 3, 0).reshape(cfg.Cin, -1))
    _pack_f32(blob, cfg.TB[0], np.stack([tr_b1, tr_b2, tr_b3], 1))
    _pack_bf16(blob, cfg.W2[0], tr_w2.transpose(1, 2, 3, 0).reshape(64, -1))
    _pack_bf16(blob, cfg.W3[0], tr_w3.transpose(1, 2, 3, 0).reshape(64, -1))
    hd = np.concatenate(
        [w.transpose(1, 2, 3, 0).reshape(64, -1) for w in (t1_w, t2_w, t3_w, t4_w, att_cw)],
        axis=1,
    )
    _pack_bf16(blob, cfg.HD[0], hd)
    _pack_f32(blob, cfg.HB[0], np.stack([t1_b, t2_b, t3_b, t4_b, att_cb], 1))
    _pack_f32(blob, cfg.AFW[0], att_fw.T)
    _pack_f32(blob, cfg.AFB[0], (att_fb / cfg.gapn)[None, :])
    _pack_f32(blob, cfg.ONE[0], np.ones((1, cfg.Cin), np.float32))
    # expertT[i, e, kh, kw, o] from expert_w[0][e, o, i, kh, kw]
    expT = expert_w[0].transpose(2, 0, 3, 4, 1).reshape(cfg.Cin, -1)
    _pack_bf16(blob, cfg.EXP[0], expT)
    return blob


def build_nc(cfg):
    nc = bacc.Bacc()
    Cin, Cout, H, W = cfg.Cin, cfg.Cout, cfg.H, cfg.W
    xin = nc.declare_dram_parameter("x", [cfg.BL, Cin, cfg.NPAD], BF16, isOutput=False)
    wblob = nc.declare_dram_parameter("wblob", [128, cfg.NBLOB], U32, isOutput=False)
    y = nc.declare_dram_parameter("y", [cfg.BL, Cout, H, W], BF16, isOutput=True)

    with tile.TileContext(nc) as tc, ExitStack() as ctx:
        cpool = ctx.enter_context(tc.tile_pool(name="consts", bufs=1))
        xpool = ctx.enter_context(tc.tile_pool(name="xpad", bufs=1))
        rpool = ctx.enter_context(tc.tile_pool(name="xrow", bufs=2))
        dpool = ctx.enter_context(tc.tile_pool(name="data", bufs=1))
        t0pool = ctx.enter_context(tc.tile_pool(name="t0p", bufs=2))
        wspool = ctx.enter_context(tc.tile_pool(name="w1sc", bufs=2))
        spool = ctx.enter_context(tc.tile_pool(name="smalls", bufs=2))
        ypool = ctx.enter_context(tc.tile_pool(name="synth", bufs=2))
        wpool = ctx.enter_context(tc.tile_pool(name="wdyn", bufs=2))
        opool = ctx.enter_context(tc.tile_pool(name="outsb", bufs=2))
        mpsum = ctx.enter_context(tc.tile_pool(name="mpsum", bufs=3, space="PSUM"))
        tpsum = ctx.enter_context(tc.tile_pool(name="tpsum", bufs=2, space="PSUM"))
        hpsum = ctx.enter_context(tc.tile_pool(name="hpsum", bufs=2, space="PSUM"))

        blob = cpool.tile([128, cfg.NBLOB], U32)
        nc.gpsimd.dma_start(blob[:, 0 : cfg.EARLY], wblob[:, 0 : cfg.EARLY])

        def bl(rng, nrows=128, dt=F32):
            ap = blob[0:nrows, rng[0]: rng[1]]
            return ap.bitcast(dt)

        wmT = bl(cfg.WMT, 32)
        wm_w1T = bl(cfg.WM1, 32)
        wm_w2T = bl(cfg.WM2)
        wm_b1 = bl(cfg.WB1)
        wm_b2 = bl(cfg.WB2)
        w1T = bl(cfg.W1, 128, BF16)
        tr_b = bl(cfg.TB, 64)
        w2T = bl(cfg.W2, 64, BF16)
        w3T = bl(cfg.W3, 64, BF16)
        headT = bl(cfg.HD, 64, BF16)
        head_b = bl(cfg.HB)
        att_fwT = bl(cfg.AFW)
        att_fb = bl(cfg.AFB, 1)
        ones_row = bl(cfg.ONE, 1)
        expT = bl(cfg.EXP, 128, BF16)
        # wm-embedding scratch (written once, read per-sample)
        wmx = cpool.tile([128, 3 * cfg.BL], F32)
        hT = wmx[:, 0 : cfg.BL]
        wmc = wmx[:, cfg.BL : 2 * cfg.BL]
        wq = wmx[:, 2 * cfg.BL : 3 * cfg.BL]

        nxp = 3
        # padded-image buffers (borders arrive pre-zeroed from the host)
        xpads = [
            xpool.tile([128, cfg.NPAD], BF16, tag=f"xp{i}", name=f"xp{i}")
            for i in range(nxp)
        ]
        xvs = [xp[:].rearrange("p (r c) -> p r c", c=cfg.Wp) for xp in xpads]

        # tower activations, column-deinterleaved (even/odd x)
        n1h = cfg.c1h * (cfg.c1w // 2)
        n2h = cfg.c2h * (cfg.c2w // 2)
        n3h = cfg.c3h * (cfg.c3w // 2)
        tower = dpool.tile([64, 2 * (n1h + n2h + n3h)], BF16, tag="tower")

        def half_views(off, n, h, w2):
            e = tower[:, off : off + n].rearrange("p (r c) -> p r c", c=w2)
            o_ = tower[:, off + n : off + 2 * n].rearrange("p (r c) -> p r c", c=w2)
            return e, o_

        t1e, t1o = half_views(0, n1h, cfg.c1h, cfg.c1w // 2)
        t2e, t2o = half_views(2 * n1h, n2h, cfg.c2h, cfg.c2w // 2)
        t3e, t3o = half_views(2 * (n1h + n2h), n3h, cfg.c3h, cfg.c3w // 2)

        # ---- wm embedding -> wm_coff.T [Cin, BL] (once, all samples) ----
        ps = hpsum.tile([128, cfg.BL], F32, tag="hps")
        nc.tensor.matmul(ps[:], wm_w1T, wmT, start=True, stop=True)
        nc.scalar.activation(hT, ps[:], AF.Prelu, bias=wm_b1, alpha=0.2)
        ps = hpsum.tile([128, cfg.BL], F32, tag="hps")
        nc.tensor.matmul(ps[:], wm_w2T, hT, start=True, stop=True)
        nc.scalar.activation(wmc, ps[:], AF.Identity, bias=wm_b2)
        nc.vector.tensor_scalar_mul(wq, wmc, 0.25)

        t0s, wdyns, w1ss = {}, {}, {}

        # ---------- per-sample stage emitters ----------
        def emit_dma(s):
            # chunked so pooling/conv can start before the full image lands
            xp = xpads[s % nxp]
            for r0, r1 in zip(cfg.dma_rows, cfg.dma_rows[1:]):
                c0, c1 = r0 * cfg.Wp, r1 * cfg.Wp
                nc.gpsimd.dma_start(xp[:, c0:c1], xin[s, :, c0:c1])

        def emit_w1s(s):
            # conv1 weights scaled by this sample's 0.25*wm_coff
            w1s = wspool.tile([128, 9 * Cout], BF16, tag="w1s", name=f"w1s_{s}")
            w1ss[s] = w1s
            nc.vector.tensor_scalar_mul(w1s[:], w1T, wq[:, s : s + 1])

        def emit_pool(s):
            t0 = t0pool.tile([128, cfg.P * cfg.PW], BF16, tag="t0",
                             name=f"t0_{s}")
            t0s[s] = t0
            t0v = t0[:].rearrange("p (r c) -> p r c", c=cfg.PW)
            xvp = xvs[s % nxp]
            pr = cfg.prows
            for k in range(cfg.NCHUNK):
                xr = rpool.tile([128, pr * cfg.Wp], BF16, tag="xr",
                                name=f"xr_{s}_{k}")
                xrv = xr[:].rearrange("p (r c) -> p r c", c=cfg.Wp)
                nc.vector.tensor_tensor(
                    xrv,
                    xvp[:, 1 + 2 * pr * k : 1 + 2 * pr * (k + 1) : 2, :],
                    xvp[:, 2 + 2 * pr * k : 2 + 2 * pr * (k + 1) : 2, :],
                    op=OP.add,
                )
                nc.vector.tensor_tensor(
                    t0v[:, pr * k : pr * (k + 1), :],
                    xrv[:, :, 1 : 2 * cfg.PW : 2],
                    xrv[:, :, 2 : 2 * cfg.PW + 1 : 2],
                    op=OP.add,
                )

        def conv_out(ps_ap, dste, dsto, nb, w2, bias):
            """psum [64, nb, 2*w2] -> lrelu+bias into even/odd col halves."""
            nc.scalar.activation(dste, ps_ap[:, :, 0 : 2 * w2 : 2], AF.Prelu,
                                 bias=bias, alpha=0.01)
            nc.scalar.activation(dsto, ps_ap[:, :, 1 : 2 * w2 : 2], AF.Prelu,
                                 bias=bias, alpha=0.01)

        def emit_conv1(s):
            t0v = t0s.pop(s)[:].rearrange("p (r c) -> p r c", c=cfg.PW)
            w1s = w1ss.pop(s)
            rb = max(1, min(cfg.c1h, 512 // cfg.c1w))
            # pairs of row-blocks stream concurrently into the two PE
            # column groups (output partitions 0-63 / 64-127)
            for y0 in range(0, cfg.c1h, 2 * rb):
                nbs = [min(rb, cfg.c1h - y0), max(0, min(rb, cfg.c1h - y0 - rb))]
                ps = tpsum.tile([128, nbs[0] * cfg.c1w], F32, tag="tps")
                for ky in range(3):
                    for kx in range(3):
                        for half, nb in enumerate(nbs):
                            if nb == 0:
                                continue
                            yh = y0 + half * rb
                            nc.tensor.matmul(
                                ps[half * 64 : half * 64 + 64, 0 : nb * cfg.c1w],
                                w1s[:, (ky * 3 + kx) * 64 : (ky * 3 + kx + 1) * 64],
                                t0v[:, yh + ky : yh + ky + nb, kx : kx + cfg.c1w],
                                start=(ky == 0 and kx == 0),
                                stop=(ky == 2 and kx == 2),
                            )
                for half, nb in enumerate(nbs):
                    if nb == 0:
                        continue
                    yh = y0 + half * rb
                    psv = ps[half * 64 : half * 64 + 64, 0 : nb * cfg.c1w]
                    psv = psv.rearrange("p (r c) -> p r c", c=cfg.c1w)
                    conv_out(psv, t1e[:, yh : yh + nb, :],
                             t1o[:, yh : yh + nb, :], nb, cfg.c1w // 2,
                             tr_b[:, 0:1])

        def emit_conv23(s):
            w2h = cfg.c2w // 2
            rb = max(1, min(cfg.c2h, 512 // cfg.c2w))
            for y0 in range(0, cfg.c2h, 2 * rb):
                nbs = [min(rb, cfg.c2h - y0), max(0, min(rb, cfg.c2h - y0 - rb))]
                ps = tpsum.tile([128, nbs[0] * cfg.c2w], F32, tag="tps")
                for ky in range(3):
                    for kx in range(3):
                        src, col0 = [(t1e, 0), (t1o, 0), (t1e, 1)][kx]
                        for half, nb in enumerate(nbs):
                            if nb == 0:
                                continue
                            yh = y0 + half * rb
                            rhs = src[:, 2 * yh + ky : 2 * yh + ky + 2 * nb : 2,
                                      col0 : col0 + cfg.c2w]
                            nc.tensor.matmul(
                                ps[half * 64 : half * 64 + 64, 0 : nb * cfg.c2w],
                                w2T[:, (ky * 3 + kx) * 64 : (ky * 3 + kx + 1) * 64],
                                rhs,
                                start=(ky == 0 and kx == 0),
                                stop=(ky == 2 and kx == 2),
                            )
                for half, nb in enumerate(nbs):
                    if nb == 0:
                        continue
                    yh = y0 + half * rb
                    psv = ps[half * 64 : half * 64 + 64, 0 : nb * cfg.c2w]
                    psv = psv.rearrange("p (r c) -> p r c", c=cfg.c2w)
                    conv_out(psv, t2e[:, yh : yh + nb, :],
                             t2o[:, yh : yh + nb, :], nb, w2h, tr_b[:, 1:2])

            ps = tpsum.tile([64, cfg.c3h * cfg.c3w], F32, tag="tps")
            for ky in range(3):
                for kx in range(3):
                    src, col0 = [(t2e, 0), (t2o, 0), (t2e, 1)][kx]
                    rhs = src[:, ky : ky + 2 * cfg.c3h - 1 : 2,
                              col0 : col0 + cfg.c3w]
                    nc.tensor.matmul(
                        ps[:],
                        w3T[:, (ky * 3 + kx) * 64 : (ky * 3 + kx + 1) * 64],
                        rhs,
                        start=(ky == 0 and kx == 0),
                        stop=(ky == 2 and kx == 2),
                    )
            psv = ps[:].rearrange("p (r c) -> p r c", c=cfg.c3w)
            conv_out(psv, t3e[:, :, :], t3o[:, :, :], cfg.c3h, cfg.c3w // 2,
                     tr_b[:, 2:3])

        def emit_heads_att_synth(s):
            sm = spool.tile([128, 64], F32, tag="sm", name=f"sm_{s}")
            a_sb = sm[:, 0:1]
            att_row = sm[0:1, 4:8]
            att_bc = sm[:, 8:12]
            cc = sm[:, 12:16]
            gap = sm[:, 16:24]
            hscr = sm[:, 24:42].bitcast(BF16)[:, 0 : cfg.gapn]
            for h in range(5):
                ps = hpsum.tile([128, cfg.gapn], F32, tag="hps")
                for ky in range(3):
                    for kx in range(3):
                        src, col0 = [(t3e, 0), (t3o, 0), (t3e, 1)][kx]
                        rhs = src[:, ky : ky + 2 * cfg.hh - 1 : 2,
                                  col0 : col0 + cfg.hw]
                        idx = h * 9 + ky * 3 + kx
                        nc.tensor.matmul(
                            ps[:],
                            headT[:, idx * 128 : (idx + 1) * 128],
                            rhs,
                            start=(ky == 0 and kx == 0),
                            stop=(ky == 2 and kx == 2),
                        )
                nc.scalar.activation(
                    hscr, ps[:], AF.Identity, bias=head_b[:, h : h + 1],
                    accum_out=gap[:, h : h + 1],
                )

            # attention: a = lrelu(gap4/gapn); att = (a@att_fwT + fb)/gapn
            nc.scalar.activation(a_sb, gap[:, 4:5], AF.Prelu,
                                 scale=1.0 / cfg.gapn, alpha=0.01)
            ps = hpsum.tile([1, 4], F32, tag="hps")
            nc.tensor.matmul(ps[:], a_sb, att_fwT, start=True, stop=True)
            nc.vector.scalar_tensor_tensor(
                att_row, ps[:], 1.0 / cfg.gapn, att_fb, op0=OP.mult, op1=OP.add
            )
            ps = hpsum.tile([128, 4], F32, tag="hps")
            nc.tensor.matmul(ps[:], ones_row, att_row, start=True, stop=True)
            nc.scalar.activation(att_bc, ps[:], AF.Copy)
            nc.vector.tensor_mul(cc, att_bc, gap[:, 0:4])

            # synthesize w_dynT[i, (kh kw o)], fold in wm_coff
            A = ypool.tile([128, 9 * 64], F32, tag="synA", name=f"synA_{s}")
            Bt = ypool.tile([128, 9 * 64], F32, tag="synB", name=f"synB_{s}")
            wdyn = wpool.tile([128, 9 * 64], BF16, tag="wdyn", name=f"wdyn_{s}")
            wdyns[s] = wdyn
            nc.vector.tensor_scalar_mul(A[:], expT[:, 0:576], cc[:, 0:1])
            nc.vector.scalar_tensor_tensor(
                Bt[:], expT[:, 576:1152], cc[:, 1:2], A[:], op0=OP.mult,
                op1=OP.add,
            )
            nc.vector.scalar_tensor_tensor(
                A[:], expT[:, 1152:1728], cc[:, 2:3], Bt[:], op0=OP.mult,
                op1=OP.add,
            )
            nc.vector.scalar_tensor_tensor(
                Bt[:], expT[:, 1728:2304], cc[:, 3:4], A[:], op0=OP.mult,
                op1=OP.add,
            )
            nc.vector.tensor_scalar_mul(wdyn[:], Bt[:], wmc[:, s : s + 1])

        def emit_main_group(s, q):
            xv = xvs[s % nxp]
            wdyn = wdyns[s]
            out_t = opool.tile([128, cfg.POUT * 512], BF16, tag="outsb",
                               name=f"out_{s}_{q}")
            for j in range(cfg.POUT):
                pair = q * cfg.POUT + j
                # even/odd row-groups stream into the two PE column groups
                ps = mpsum.tile([128, 512], F32, tag="mps")
                for ky in range(3):
                    for kx in range(3):
                        for half in range(2):
                            y0 = (2 * pair + half) * cfg.RPG
                            nc.tensor.matmul(
                                ps[half * 64 : half * 64 + 64, :],
                                wdyn[:, (ky * 3 + kx) * 64 : (ky * 3 + kx + 1) * 64],
                                xv[:, y0 + ky : y0 + ky + cfg.RPG,
                                   kx : kx + cfg.W],
                                start=(ky == 0 and kx == 0),
                                stop=(ky == 2 and kx == 2),
                            )
                nc.vector.tensor_copy(out_t[:, j * 512 : (j + 1) * 512], ps[:])
            yv = y[s].rearrange("c (j r) x -> c j r x", r=2 * cfg.RPG)
            jj = q * cfg.POUT
            for hf in range(2):
                dst = yv[:, jj : jj + cfg.POUT,
                         hf * cfg.RPG : (hf + 1) * cfg.RPG, :]
                nc.sync.dma_start(dst, out_t[hf * 64 : hf * 64 + 64, :])
            if q == cfg.NOUT - 1:
                wdyns.pop(s)

        # ---------- software pipeline ----------
        # prologue: sample 0 (and 1) fully up to synth before main(0)
        emit_dma(0)
        emit_w1s(0)
        emit_pool(0)
        emit_conv1(0)
        for c0, c1 in [(cfg.EARLY, cfg.MID1), (cfg.MID1, cfg.MID2),
                       (cfg.MID2, cfg.MID), (cfg.MID, cfg.NBLOB)]:
            nc.gpsimd.dma_start(blob[:, c0:c1], wblob[:, c0:c1])
        if cfg.BL > 1:
            emit_dma(1)
        emit_conv23(0)
        emit_heads_att_synth(0)
        if cfg.BL > 1:
            emit_w1s(1)
            emit_pool(1)

        # stage k of sample s+1 (or s+2 for dma/pool) after main group q=k
        def stage_after(s, q):
            if q == min(0, cfg.NOUT - 1):
                if s + 2 < cfg.BL:
                    emit_dma(s + 2)
                if s + 1 < cfg.BL:
                    emit_conv1(s + 1)
            if q == min(1, cfg.NOUT - 1):
                if s + 1 < cfg.BL:
                    emit_conv23(s + 1)
            if q == min(2, cfg.NOUT - 1):
                if s + 1 < cfg.BL:
                    emit_heads_att_synth(s + 1)
                if s + 2 < cfg.BL:
                    emit_w1s(s + 2)
            if q == cfg.NOUT - 1:
                if s + 2 < cfg.BL:
                    emit_pool(s + 2)

        for s in range(cfg.BL):
            for q in range(cfg.NOUT):
                emit_main_group(s, q)
                stage_after(s, q)

    return nc


_NC_CACHE = {}
TRACE = False       # set by test harness to collect an NTFF profile
TRACE_DIR = None    # where to leave the NTFF/perfetto artifacts
LAST_RESULT = None  # BassKernelResults of the most recent kernel() call


def _get_nc(cfg):
    key = (cfg.BL, cfg.Cin, cfg.H, cfg.W)
    if key not in _NC_CACHE:
        nc = build_nc(cfg)
        if not nc.is_finalized():
            nc.finalize()
        _NC_CACHE[key] = nc
    return _NC_CACHE[key]


def pad_images(cfg, x):
    """[n, Cin, H, W] -> zero-padded flat [n, Cin, Hp*Wp] bf16."""
    n = x.shape[0]
    xp = np.zeros((n, cfg.Cin, cfg.Hp, cfg.Wp), ml_dtypes.bfloat16)
    xp[:, :, 1 : cfg.H + 1, 1 : cfg.W + 1] = x.astype(ml_dtypes.bfloat16)
    return xp.reshape(n, cfg.Cin, cfg.NPAD)


def kernel(**inputs):
    x = np.asarray(inputs["x"], np.float32)
    B, Cin, H, W = x.shape
    cfg = Cfg(BL=B // 8, Cin=Cin, H=H, W=W)
    nc = _get_nc(cfg)
    wnames = [
        "wm_w1", "wm_b1", "wm_w2", "wm_b2", "tr_w1", "tr_b1", "tr_w2", "tr_b2",
        "tr_w3", "tr_b3", "t1_w", "t1_b", "t2_w", "t2_b", "t3_w", "t3_b",
        "t4_w", "t4_b", "att_cw", "att_cb", "att_fw", "att_fb", "expert_w",
    ]
    ws = {k: np.asarray(inputs[k], np.float32) for k in wnames}
    wm = np.asarray(inputs["wm"], np.float32)
    in_maps = []
    for c in range(8):
        sl = slice(c * cfg.BL, (c + 1) * cfg.BL)
        blob = make_blob(cfg, wm[sl], **ws)
        in_maps.append({"x": pad_images(cfg, x[sl]), "wblob": blob})
    global LAST_RESULT
    kw = {"tmpdir": TRACE_DIR} if (TRACE and TRACE_DIR) else {}
    res = run_bass_kernel_spmd(nc, in_maps, list(range(8)), trace=TRACE, **kw)
    LAST_RESULT = res
    return np.concatenate(
        [res.results[c]["y"].astype(np.float32) for c in range(8)], axis=0
    )


# revision 15
# speedup vs baseline: 1.3050x; 1.0253x over previous
"""Trainium2 Bass kernel for nn_CustomConvLayer (dynamic per-sample conv).

Sharding: pure data parallel over batch B=32 across 8 NeuronCores (4
samples per core). Small synthesis networks are replicated per core.

Per sample, on device:
  wm-embedding MLP -> per-channel modulation wm_coff  (tiny matmuls)
  t0 = avg_pool2(x)                                   (two DVE tensor_tensor
                                                       adds: vertical pair sum
                                                       in 2x bf16 mode, then
                                                       strided horizontal sum)
  tower: 3 convs (ACT-fused lrelu) -> t3              (9-tap matmul conv, bf16;
                                                       wm_coff*0.25 folded into
                                                       per-sample conv1 weights)
  4 coeff heads + attention head (conv+GAP)           (bf16 matmuls + accum)
  w_dyn synthesis (4 experts)                         (DVE scalar_tensor_tensor)
  main conv: out = conv(x, w_dyn*wm_coff), 3x3 pad 1  (bf16 matmuls, PSUM accum)
Tower activations t1/t2/t3 are stored column-deinterleaved (even/odd x)
so the stride-2 convs read contiguous rhs. Output y is written bf16 and
upcast to f32 on the host.
"""

import sys

if "/opt/trn_rl_repo" not in sys.path:
    sys.path.insert(0, "/opt/trn_rl_repo")

import numpy as np
import ml_dtypes
from contextlib import ExitStack

import concourse.bass as bass
import concourse.bacc as bacc
import concourse.tile as tile
from concourse import mybir
from concourse.bass_utils import run_bass_kernel_spmd

F32 = mybir.dt.float32
BF16 = mybir.dt.bfloat16
U32 = mybir.dt.uint32
AF = mybir.ActivationFunctionType
OP = mybir.AluOpType


class Cfg:
    def __init__(self, BL=4, Cin=128, H=128, W=128, n_cores=8):
        self.BL, self.Cin, self.H, self.W, self.n_cores = BL, Cin, H, W, n_cores
        self.Cout = 64
        self.Hp, self.Wp = H + 2, W + 2
        self.NPAD = self.Hp * self.Wp
        # pooled size and tower conv output sizes
        self.P, self.PW = H // 2, W // 2
        self.c1h, self.c1w = self.P - 2, self.PW - 2
        self.c2h, self.c2w = (self.c1h - 3) // 2 + 1, (self.c1w - 3) // 2 + 1
        self.c3h, self.c3w = (self.c2h - 3) // 2 + 1, (self.c2w - 3) // 2 + 1
        self.hh, self.hw = (self.c3h - 3) // 2 + 1, (self.c3w - 3) // 2 + 1
        self.gapn = self.hh * self.hw
        # x DMA / pooling chunk rows (padded-image rows per chunk)
        self.NCHUNK = 4
        self.prows = self.P // self.NCHUNK          # pooled rows per chunk
        # dma chunks must cover pool chunk k's rows [2*prows*k+1, +2*prows+1)
        r0 = 2 * self.prows + 2
        self.dma_rows = [0, r0]
        while self.dma_rows[-1] < self.Hp:
            self.dma_rows.append(min(self.Hp, self.dma_rows[-1] + 2 * self.prows))
        # main conv row groups: pairs of row-groups share one PSUM bank
        self.RPG = 512 // W          # rows per row-group (N = RPG*W = 512)
        assert H % (2 * self.RPG) == 0
        self.NRG = H // self.RPG
        self.NPAIR = self.NRG // 2
        self.POUT = min(4, self.NPAIR)   # psum-pairs per output staging tile
        assert self.NPAIR % self.POUT == 0
        self.NOUT = self.NPAIR // self.POUT
        # ---- const blob layout (uint32 columns) ----
        o = 0

        def take(n):
            nonlocal o
            r = (o, o + n)
            o += n
            return r

        # early: wm embedding + conv1 weights
        self.WMT = take(BL)                       # wm.T f32 [32, BL]
        self.WM1 = take(self.Cin)                 # wm_w1T f32 [32, 128]
        self.WM2 = take(self.Cin)                 # wm_w2T f32 [128, 128]
        self.WB1 = take(1)                        # wm_b1 f32 [128, 1]
        self.WB2 = take(1)                        # wm_b2 f32 [128, 1]
        self.W1 = take(9 * self.Cout // 2)        # w1T bf16 [128, 9*64]
        self.TB = take(3)                         # tower biases f32 [64, 3]
        self.EARLY = o
        # mid: rest of the tower + heads
        self.W2 = take(9 * self.Cout // 2)        # w2T bf16 [64, 9*64]
        self.W3 = take(9 * self.Cout // 2)        # w3T bf16 [64, 9*64]
        self.MID1 = o
        self.HD = take(5 * 9 * self.Cin // 2)     # headT bf16 [64, 5*9*128]
        self.MID2 = self.HD[0] + 2 * 9 * self.Cin // 2   # after heads 0-1
        self.HB = take(5)                         # head biases f32 [128, 5]
        self.AFW = take(4)                        # att_fw.T f32 [128, 4]
        self.AFB = take(4)                        # att_fb/gapn f32 [1, 4]
        self.ONE = take(self.Cin)                 # ones f32 [1, 128]
        self.MID = o
        # late: expert weights
        self.EXP = take(4 * 9 * self.Cout // 2)   # expertT bf16 [128, 4*9*64]
        self.NBLOB = o


def _pack_f32(dst, col, arr):
    """Pack f32 array [r, c] into dst u32 blob at column col."""
    a = np.ascontiguousarray(arr, dtype=np.float32)
    dst[: a.shape[0], col : col + a.shape[1]] = a.view(np.uint32)


def _pack_bf16(dst, col, arr):
    a = np.ascontiguousarray(arr, dtype=ml_dtypes.bfloat16)
    u16 = a.view(np.uint16)
    u32 = (u16[:, 1::2].astype(np.uint32) << 16) | u16[:, 0::2].astype(np.uint32)
    dst[: a.shape[0], col : col + u32.shape[1]] = u32


def make_blob(cfg, wm_core, wm_w1, wm_b1, wm_w2, wm_b2, tr_w1, tr_b1, tr_w2,
              tr_b2, tr_w3, tr_b3, t1_w, t1_b, t2_w, t2_b, t3_w, t3_b, t4_w,
              t4_b, att_cw, att_cb, att_fw, att_fb, expert_w):
    """Pack all weights + this core's wm rows into one [128, NBLOB] u32 blob."""
    blob = np.zeros((128, cfg.NBLOB), np.uint32)
    _pack_f32(blob, cfg.WMT[0], wm_core.T)
    _pack_f32(blob, cfg.WM1[0], wm_w1.T)
    _pack_f32(blob, cfg.WM2[0], wm_w2.T)
    _pack_f32(blob, cfg.WB1[0], wm_b1[:, None])
    _pack_f32(blob, cfg.WB2[0], wm_b2[:, None])
    # conv weights [o, i, kh, kw] -> [i, kh, kw, o]
    _pack_bf16(blob, cfg.W1[0], tr_w1.transpose(1, 2, 3, 0).reshape(cfg.Cin, -1))
    _pack_f32(blob, cfg.TB[0], np.stack([tr_b1, tr_b2, tr_b3], 1))
    _pack_bf16(blob, cfg.W2[0], tr_w2.transpose(1, 2, 3, 0).reshape(64, -1))
    _pack_bf16(blob, cfg.W3[0], tr_w3.transpose(1, 2, 3, 0).reshape(64, -1))
    hd = np.concatenate(
        [w.transpose(1, 2, 3, 0).reshape(64, -1) for w in (t1_w, t2_w, t3_w, t4_w, att_cw)],
        axis=1,
    )
    _pack_bf16(blob, cfg.HD[0], hd)
    # duplicate head weights into partitions 64-127 (rides along in the
    # same DMA): lets head taps alternate PE row groups (K=64 row tiling)
    _pack_bf16(blob[64:128], cfg.HD[0], hd)
    _pack_f32(blob, cfg.HB[0], np.stack([t1_b, t2_b, t3_b, t4_b, att_cb], 1))
    _pack_f32(blob, cfg.AFW[0], att_fw.T)
    _pack_f32(blob, cfg.AFB[0], (att_fb / cfg.gapn)[None, :])
    _pack_f32(blob, cfg.ONE[0], np.ones((1, cfg.Cin), np.float32))
    # expertT[i, e, kh, kw, o] from expert_w[0][e, o, i, kh, kw]
    expT = expert_w[0].transpose(2, 0, 3, 4, 1).reshape(cfg.Cin, -1)
    _pack_bf16(blob, cfg.EXP[0], expT)
    return blob


def build_nc(cfg):
    nc = bacc.Bacc()
    Cin, Cout, H, W = cfg.Cin, cfg.Cout, cfg.H, cfg.W
    xin = nc.declare_dram_parameter("x", [cfg.BL, Cin, cfg.NPAD], BF16, isOutput=False)
    wblob = nc.declare_dram_parameter("wblob", [128, cfg.NBLOB], U32, isOutput=False)
    y = nc.declare_dram_parameter("y", [cfg.BL, Cout, H, W], BF16, isOutput=True)

    with tile.TileContext(nc) as tc, ExitStack() as ctx:
        cpool = ctx.enter_context(tc.tile_pool(name="consts", bufs=1))
        xpool = ctx.enter_context(tc.tile_pool(name="xpad", bufs=1))
        rpool = ctx.enter_context(tc.tile_pool(name="xrow", bufs=2))
        dpool = ctx.enter_context(tc.tile_pool(name="data", bufs=1))
        t0pool = ctx.enter_context(tc.tile_pool(name="t0p", bufs=2))
        wspool = ctx.enter_context(tc.tile_pool(name="w1sc", bufs=2))
        spool = ctx.enter_context(tc.tile_pool(name="smalls", bufs=2))
        ypool = ctx.enter_context(tc.tile_pool(name="synth", bufs=2))
        wpool = ctx.enter_context(tc.tile_pool(name="wdyn", bufs=2))
        opool = ctx.enter_context(tc.tile_pool(name="outsb", bufs=2))
        mpsum = ctx.enter_context(tc.tile_pool(name="mpsum", bufs=4, space="PSUM"))
        tpsum = ctx.enter_context(tc.tile_pool(name="tpsum", bufs=2, space="PSUM"))
        hpsum = ctx.enter_context(tc.tile_pool(name="hpsum", bufs=2, space="PSUM"))

        blob = cpool.tile([128, cfg.NBLOB], U32)
        nc.gpsimd.dma_start(blob[:, 0 : cfg.EARLY], wblob[:, 0 : cfg.EARLY])

        def bl(rng, nrows=128, dt=F32):
            ap = blob[0:nrows, rng[0]: rng[1]]
            return ap.bitcast(dt)

        wmT = bl(cfg.WMT, 32)
        wm_w1T = bl(cfg.WM1, 32)
        wm_w2T = bl(cfg.WM2)
        wm_b1 = bl(cfg.WB1)
        wm_b2 = bl(cfg.WB2)
        w1T = bl(cfg.W1, 128, BF16)
        tr_b = bl(cfg.TB, 64)
        w2T = bl(cfg.W2, 64, BF16)
        w3T = bl(cfg.W3, 64, BF16)
        headT = bl(cfg.HD, 64, BF16)
        headT_hi = blob[64:128, cfg.HD[0] : cfg.HD[1]].bitcast(BF16)
        head_b = bl(cfg.HB)
        att_fwT = bl(cfg.AFW)
        att_fb = bl(cfg.AFB, 1)
        ones_row = bl(cfg.ONE, 1)
        expT = bl(cfg.EXP, 128, BF16)
        # wm-embedding scratch (written once, read per-sample)
        wmx = cpool.tile([128, 3 * cfg.BL], F32)
        hT = wmx[:, 0 : cfg.BL]
        wmc = wmx[:, cfg.BL : 2 * cfg.BL]
        wq = wmx[:, 2 * cfg.BL : 3 * cfg.BL]

        nxp = 3
        # padded-image buffers (borders arrive pre-zeroed from the host)
        xpads = [
            xpool.tile([128, cfg.NPAD], BF16, tag=f"xp{i}", name=f"xp{i}")
            for i in range(nxp)
        ]
        xvs = [xp[:].rearrange("p (r c) -> p r c", c=cfg.Wp) for xp in xpads]

        # tower activations, column-deinterleaved (even/odd x); t3 is
        # duplicated on partitions 64-127 so head taps can row-tile
        n1h = cfg.c1h * (cfg.c1w // 2)
        n2h = cfg.c2h * (cfg.c2w // 2)
        n3h = cfg.c3h * (cfg.c3w // 2)
        tower = dpool.tile([128, 2 * (n1h + n2h + n3h)], BF16, tag="tower")

        def half_views(off, n, w2, p0=0):
            t = tower[p0 : p0 + 64, :]
            e = t[:, off : off + n].rearrange("p (r c) -> p r c", c=w2)
            o_ = t[:, off + n : off + 2 * n].rearrange("p (r c) -> p r c", c=w2)
            return e, o_

        t1e, t1o = half_views(0, n1h, cfg.c1w // 2)
        t2e, t2o = half_views(2 * n1h, n2h, cfg.c2w // 2)
        t3off = 2 * (n1h + n2h)
        t3e, t3o = half_views(t3off, n3h, cfg.c3w // 2)
        t3eh, t3oh = half_views(t3off, n3h, cfg.c3w // 2, p0=64)

        # ---- wm embedding -> wm_coff.T [Cin, BL] (once, all samples) ----
        ps = hpsum.tile([128, cfg.BL], F32, tag="hps")
        nc.tensor.matmul(ps[:], wm_w1T, wmT, start=True, stop=True)
        nc.scalar.activation(hT, ps[:], AF.Prelu, bias=wm_b1, alpha=0.2)
        ps = hpsum.tile([128, cfg.BL], F32, tag="hps")
        nc.tensor.matmul(ps[:], wm_w2T, hT, start=True, stop=True)
        nc.scalar.activation(wmc, ps[:], AF.Identity, bias=wm_b2)
        nc.vector.tensor_scalar_mul(wq, wmc, 0.25)

        t0s, wdyns, w1ss = {}, {}, {}

        # ---------- per-sample stage emitters ----------
        def emit_dma(s):
            # chunked so pooling/conv can start before the full image lands
            xp = xpads[s % nxp]
            for r0, r1 in zip(cfg.dma_rows, cfg.dma_rows[1:]):
                c0, c1 = r0 * cfg.Wp, r1 * cfg.Wp
                nc.gpsimd.dma_start(xp[:, c0:c1], xin[s, :, c0:c1])

        def emit_w1s(s):
            # conv1 weights scaled by this sample's 0.25*wm_coff
            w1s = wspool.tile([128, 9 * Cout], BF16, tag="w1s", name=f"w1s_{s}")
            w1ss[s] = w1s
            nc.vector.tensor_scalar_mul(w1s[:], w1T, wq[:, s : s + 1])

        def emit_pool(s):
            t0 = t0pool.tile([128, cfg.P * cfg.PW], BF16, tag="t0",
                             name=f"t0_{s}")
            t0s[s] = t0
            t0v = t0[:].rearrange("p (r c) -> p r c", c=cfg.PW)
            xvp = xvs[s % nxp]
            pr = cfg.prows
            for k in range(cfg.NCHUNK):
                xr = rpool.tile([128, pr * cfg.Wp], BF16, tag="xr",
                                name=f"xr_{s}_{k}")
                xrv = xr[:].rearrange("p (r c) -> p r c", c=cfg.Wp)
                nc.vector.tensor_tensor(
                    xrv,
                    xvp[:, 1 + 2 * pr * k : 1 + 2 * pr * (k + 1) : 2, :],
                    xvp[:, 2 + 2 * pr * k : 2 + 2 * pr * (k + 1) : 2, :],
                    op=OP.add,
                )
                nc.vector.tensor_tensor(
                    t0v[:, pr * k : pr * (k + 1), :],
                    xrv[:, :, 1 : 2 * cfg.PW : 2],
                    xrv[:, :, 2 : 2 * cfg.PW + 1 : 2],
                    op=OP.add,
                )

        def conv_out(ps_ap, dste, dsto, nb, w2, bias):
            """psum [64, nb, 2*w2] -> lrelu+bias into even/odd col halves."""
            nc.scalar.activation(dste, ps_ap[:, :, 0 : 2 * w2 : 2], AF.Prelu,
                                 bias=bias, alpha=0.01)
            nc.scalar.activation(dsto, ps_ap[:, :, 1 : 2 * w2 : 2], AF.Prelu,
                                 bias=bias, alpha=0.01)

        def emit_conv1(s):
            t0v = t0s.pop(s)[:].rearrange("p (r c) -> p r c", c=cfg.PW)
            w1s = w1ss.pop(s)
            rb = max(1, min(cfg.c1h, 512 // cfg.c1w))
            # pairs of row-blocks stream concurrently into the two PE
            # column groups (output partitions 0-63 / 64-127)
            for y0 in range(0, cfg.c1h, 2 * rb):
                nbs = [min(rb, cfg.c1h - y0), max(0, min(rb, cfg.c1h - y0 - rb))]
                ps = tpsum.tile([128, nbs[0] * cfg.c1w], F32, tag="tps")
                for ky in range(3):
                    for kx in range(3):
                        for half, nb in enumerate(nbs):
                            if nb == 0:
                                continue
                            yh = y0 + half * rb
                            nc.tensor.matmul(
                                ps[half * 64 : half * 64 + 64, 0 : nb * cfg.c1w],
                                w1s[:, (ky * 3 + kx) * 64 : (ky * 3 + kx + 1) * 64],
                                t0v[:, yh + ky : yh + ky + nb, kx : kx + cfg.c1w],
                                start=(ky == 0 and kx == 0),
                                stop=(ky == 2 and kx == 2),
                            )
                for half, nb in enumerate(nbs):
                    if nb == 0:
                        continue
                    yh = y0 + half * rb
                    psv = ps[half * 64 : half * 64 + 64, 0 : nb * cfg.c1w]
                    psv = psv.rearrange("p (r c) -> p r c", c=cfg.c1w)
                    conv_out(psv, t1e[:, yh : yh + nb, :],
                             t1o[:, yh : yh + nb, :], nb, cfg.c1w // 2,
                             tr_b[:, 0:1])

        def emit_conv23(s):
            w2h = cfg.c2w // 2
            rb = max(1, min(cfg.c2h, 512 // cfg.c2w))
            for y0 in range(0, cfg.c2h, 2 * rb):
                nbs = [min(rb, cfg.c2h - y0), max(0, min(rb, cfg.c2h - y0 - rb))]
                ps = tpsum.tile([128, nbs[0] * cfg.c2w], F32, tag="tps")
                for ky in range(3):
                    for kx in range(3):
                        src, col0 = [(t1e, 0), (t1o, 0), (t1e, 1)][kx]
                        for half, nb in enumerate(nbs):
                            if nb == 0:
                                continue
                            yh = y0 + half * rb
                            rhs = src[:, 2 * yh + ky : 2 * yh + ky + 2 * nb : 2,
                                      col0 : col0 + cfg.c2w]
                            nc.tensor.matmul(
                                ps[half * 64 : half * 64 + 64, 0 : nb * cfg.c2w],
                                w2T[:, (ky * 3 + kx) * 64 : (ky * 3 + kx + 1) * 64],
                                rhs,
                                start=(ky == 0 and kx == 0),
                                stop=(ky == 2 and kx == 2),
                            )
                for half, nb in enumerate(nbs):
                    if nb == 0:
                        continue
                    yh = y0 + half * rb
                    psv = ps[half * 64 : half * 64 + 64, 0 : nb * cfg.c2w]
                    psv = psv.rearrange("p (r c) -> p r c", c=cfg.c2w)
                    conv_out(psv, t2e[:, yh : yh + nb, :],
                             t2o[:, yh : yh + nb, :], nb, w2h, tr_b[:, 1:2])

            # conv3 computed twice (both PE column groups, concurrent) so t3
            # lands on partitions 0-63 AND 64-127 for head row-tiling
            ps = tpsum.tile([128, cfg.c3h * cfg.c3w], F32, tag="tps")
            for ky in range(3):
                for kx in range(3):
                    src, col0 = [(t2e, 0), (t2o, 0), (t2e, 1)][kx]
                    rhs = src[:, ky : ky + 2 * cfg.c3h - 1 : 2,
                              col0 : col0 + cfg.c3w]
                    for half in range(2):
                        nc.tensor.matmul(
                            ps[half * 64 : half * 64 + 64, :],
                            w3T[:, (ky * 3 + kx) * 64 : (ky * 3 + kx + 1) * 64],
                            rhs,
                            start=(ky == 0 and kx == 0),
                            stop=(ky == 2 and kx == 2),
                        )
            psv = ps[0:64, :].rearrange("p (r c) -> p r c", c=cfg.c3w)
            conv_out(psv, t3e[:, :, :], t3o[:, :, :], cfg.c3h, cfg.c3w // 2,
                     tr_b[:, 2:3])
            psvh = ps[64:128, :].rearrange("p (r c) -> p r c", c=cfg.c3w)
            conv_out(psvh, t3eh[:, :, :], t3oh[:, :, :], cfg.c3h,
                     cfg.c3w // 2, tr_b[:, 2:3])

        def emit_heads_att_synth(s):
            sm = spool.tile([128, 64], F32, tag="sm", name=f"sm_{s}")
            a_sb = sm[:, 0:1]
            att_row = sm[0:1, 4:8]
            att_bc = sm[:, 8:12]
            cc = sm[:, 12:16]
            gap = sm[:, 16:24]
            hscr = sm[:, 24:42].bitcast(BF16)[:, 0 : cfg.gapn]
            for h in range(5):
                ps = hpsum.tile([128, cfg.gapn], F32, tag="hps")
                for ky in range(3):
                    for kx in range(3):
                        src, col0 = [(t3e, 0), (t3o, 0), (t3e, 1)][kx]
                        rhs = src[:, ky : ky + 2 * cfg.hh - 1 : 2,
                                  col0 : col0 + cfg.hw]
                        idx = h * 9 + ky * 3 + kx
                        nc.tensor.matmul(
                            ps[:],
                            headT[:, idx * 128 : (idx + 1) * 128],
                            rhs,
                            start=(ky == 0 and kx == 0),
                            stop=(ky == 2 and kx == 2),
                        )
                nc.scalar.activation(
                    hscr, ps[:], AF.Identity, bias=head_b[:, h : h + 1],
                    accum_out=gap[:, h : h + 1],
                )

            # attention: a = lrelu(gap4/gapn); att = (a@att_fwT + fb)/gapn
            nc.scalar.activation(a_sb, gap[:, 4:5], AF.Prelu,
                                 scale=1.0 / cfg.gapn, alpha=0.01)
            ps = hpsum.tile([1, 4], F32, tag="hps")
            nc.tensor.matmul(ps[:], a_sb, att_fwT, start=True, stop=True)
            nc.vector.scalar_tensor_tensor(
                att_row, ps[:], 1.0 / cfg.gapn, att_fb, op0=OP.mult, op1=OP.add
            )
            ps = hpsum.tile([128, 4], F32, tag="hps")
            nc.tensor.matmul(ps[:], ones_row, att_row, start=True, stop=True)
            nc.scalar.activation(att_bc, ps[:], AF.Copy)
            nc.vector.tensor_mul(cc, att_bc, gap[:, 0:4])

            # synthesize w_dynT[i, (kh kw o)], fold in wm_coff
            A = ypool.tile([128, 9 * 64], F32, tag="synA", name=f"synA_{s}")
            Bt = ypool.tile([128, 9 * 64], F32, tag="synB", name=f"synB_{s}")
            wdyn = wpool.tile([128, 9 * 64], BF16, tag="wdyn", name=f"wdyn_{s}")
            wdyns[s] = wdyn
            nc.vector.tensor_scalar_mul(A[:], expT[:, 0:576], cc[:, 0:1])
            nc.vector.scalar_tensor_tensor(
                Bt[:], expT[:, 576:1152], cc[:, 1:2], A[:], op0=OP.mult,
                op1=OP.add,
            )
            nc.vector.scalar_tensor_tensor(
                A[:], expT[:, 1152:1728], cc[:, 2:3], Bt[:], op0=OP.mult,
                op1=OP.add,
            )
            nc.vector.scalar_tensor_tensor(
                Bt[:], expT[:, 1728:2304], cc[:, 3:4], A[:], op0=OP.mult,
                op1=OP.add,
            )
            nc.vector.tensor_scalar_mul(wdyn[:], Bt[:], wmc[:, s : s + 1])

        def emit_main_group(s, q):
            xv = xvs[s % nxp]
            wdyn = wdyns[s]
            out_t = opool.tile([128, cfg.POUT * 512], BF16, tag="outsb",
                               name=f"out_{s}_{q}")
            for j in range(cfg.POUT):
                pair = q * cfg.POUT + j
                # even/odd row-groups stream into the two PE column groups
                ps = mpsum.tile([128, 512], F32, tag="mps")
                for ky in range(3):
                    for kx in range(3):
                        for half in range(2):
                            y0 = (2 * pair + half) * cfg.RPG
                            nc.tensor.matmul(
                                ps[half * 64 : half * 64 + 64, :],
                                wdyn[:, (ky * 3 + kx) * 64 : (ky * 3 + kx + 1) * 64],
                                xv[:, y0 + ky : y0 + ky + cfg.RPG,
                                   kx : kx + cfg.W],
                                start=(ky == 0 and kx == 0),
                                stop=(ky == 2 and kx == 2),
                            )
                dst_o = out_t[:, j * 512 : (j + 1) * 512]
                if j % 2 == 0:
                    nc.scalar.activation(dst_o, ps[:], AF.Copy)
                else:
                    nc.vector.tensor_copy(dst_o, ps[:])
            yv = y[s].rearrange("c (j r) x -> c j r x", r=2 * cfg.RPG)
            jj = q * cfg.POUT
            last = s == cfg.BL - 1 and q == cfg.NOUT - 1
            nj = 2 if last else cfg.POUT   # finer drain on the final group
            for j0 in range(0, cfg.POUT, nj):
                for hf in range(2):
                    dst = yv[:, jj + j0 : jj + j0 + nj,
                             hf * cfg.RPG : (hf + 1) * cfg.RPG, :]
                    src = out_t[hf * 64 : hf * 64 + 64,
                                j0 * 512 : (j0 + nj) * 512]
                    nc.sync.dma_start(dst, src)
            if q == cfg.NOUT - 1:
                wdyns.pop(s)

        # ---------- software pipeline ----------
        # prologue: sample 0 (and 1) fully up to synth before main(0)
        emit_dma(0)
        emit_w1s(0)
        emit_pool(0)
        emit_conv1(0)
        for c0, c1 in [(cfg.EARLY, cfg.MID1), (cfg.MID1, cfg.MID2),
                       (cfg.MID2, cfg.MID), (cfg.MID, cfg.NBLOB)]:
            nc.gpsimd.dma_start(blob[:, c0:c1], wblob[:, c0:c1])
        if cfg.BL > 1:
            emit_dma(1)
        emit_conv23(0)
        emit_heads_att_synth(0)
        if cfg.BL > 1:
            emit_w1s(1)
            emit_pool(1)

        # stage k of sample s+1 (or s+2 for dma/pool) after main group q=k
        def stage_after(s, q):
            if q == min(0, cfg.NOUT - 1):
                if s + 2 < cfg.BL:
                    emit_dma(s + 2)
                if s + 1 < cfg.BL:
                    emit_conv1(s + 1)
            if q == min(1, cfg.NOUT - 1):
                if s + 1 < cfg.BL:
                    emit_conv23(s + 1)
            if q == min(2, cfg.NOUT - 1):
                if s + 1 < cfg.BL:
                    emit_heads_att_synth(s + 1)
                if s + 2 < cfg.BL:
                    emit_w1s(s + 2)
            if q == cfg.NOUT - 1:
                if s + 2 < cfg.BL:
                    emit_pool(s + 2)

        for s in range(cfg.BL):
            for q in range(cfg.NOUT):
                emit_main_group(s, q)
                stage_after(s, q)

    return nc


_NC_CACHE = {}
TRACE = False       # set by test harness to collect an NTFF profile
TRACE_DIR = None    # where to leave the NTFF/perfetto artifacts
LAST_RESULT = None  # BassKernelResults of the most recent kernel() call


def _get_nc(cfg):
    key = (cfg.BL, cfg.Cin, cfg.H, cfg.W)
    if key not in _NC_CACHE:
        nc = build_nc(cfg)
        if not nc.is_finalized():
            nc.finalize()
        _NC_CACHE[key] = nc
    return _NC_CACHE[key]


def pad_images(cfg, x):
    """[n, Cin, H, W] -> zero-padded flat [n, Cin, Hp*Wp] bf16."""
    n = x.shape[0]
    xp = np.zeros((n, cfg.Cin, cfg.Hp, cfg.Wp), ml_dtypes.bfloat16)
    xp[:, :, 1 : cfg.H + 1, 1 : cfg.W + 1] = x.astype(ml_dtypes.bfloat16)
    return xp.reshape(n, cfg.Cin, cfg.NPAD)


def kernel(**inputs):
    x = np.asarray(inputs["x"], np.float32)
    B, Cin, H, W = x.shape
    cfg = Cfg(BL=B // 8, Cin=Cin, H=H, W=W)
    nc = _get_nc(cfg)
    wnames = [
        "wm_w1", "wm_b1", "wm_w2", "wm_b2", "tr_w1", "tr_b1", "tr_w2", "tr_b2",
        "tr_w3", "tr_b3", "t1_w", "t1_b", "t2_w", "t2_b", "t3_w", "t3_b",
        "t4_w", "t4_b", "att_cw", "att_cb", "att_fw", "att_fb", "expert_w",
    ]
    ws = {k: np.asarray(inputs[k], np.float32) for k in wnames}
    wm = np.asarray(inputs["wm"], np.float32)
    in_maps = []
    for c in range(8):
        sl = slice(c * cfg.BL, (c + 1) * cfg.BL)
        blob = make_blob(cfg, wm[sl], **ws)
        in_maps.append({"x": pad_images(cfg, x[sl]), "wblob": blob})
    global LAST_RESULT
    kw = {"tmpdir": TRACE_DIR} if (TRACE and TRACE_DIR) else {}
    res = run_bass_kernel_spmd(nc, in_maps, list(range(8)), trace=TRACE, **kw)
    LAST_RESULT = res
    return np.concatenate(
        [res.results[c]["y"].astype(np.float32) for c in range(8)], axis=0
    )
